# revision 47
# baseline (speedup 1.0000x reference)
"""Gemma2 sliding-window attention on 8 Trainium2 NeuronCores.

Sharding: data-parallel over batch (4) x tensor-parallel over heads (2).
Core c handles batch b = c//2 and head-half hh = c%2 (4 of 8 q-heads,
2 of 4 kv-heads). Each core computes a partial output [S, H] =
attn(local heads) @ Wo[local rows, :]; host sums the two partials per batch.

Device layout choice: everything is computed in "transposed" orientation.
hidden is fed as hsT [H, S]; projections produce qT/kT [d, s] and V [s, d];
scores are computed transposed sT [k, q] so the softmax denominator comes
from cross-partition reduction; PV produces attnT [d, q] which feeds the
O-projection as the stationary operand, yielding out [s, h] directly.

Optimizations vs the original baseline (519us -> ~456us):
 - softmax denominators mostly moved OFF the PE: eT tiles are accumulated
   into a per-group esum on the DVE (partial-width adds mirroring the old
   per-k-tile ones-matmul widths), leaving a single N=512 ones-matmul per
   group (~47k PE columns saved), then DVE reciprocal + gpsimd broadcast.
 - phase 2 is a single flattened (h,g,k-tile) work stream with a 5-deep
   scores-ahead-of-PV pipeline, so the scores->exp->mask chain latency is
   hidden and group boundaries don't bubble the PE. The sc_ps ring is 4
   banks (sums share the "big" ring, using partition row 0 only).
 - the group-finish chain (ones-matmul -> recip -> gpsimd broadcast ->
   normalize muls) is staged over deferred work items (delays 1/2/4 pops)
   so the in-order DVE queue never head-of-line blocks on gpsimd, while
   group G's normalize muls still land before group G+2 reuses its psum
   accumulator banks (the PV matmuls use skip_group_check, so only
   program order protects that WAR).
 - gpsimd partition_broadcast is warmed up during phase 1; its first
   invocation otherwise pays a ~6us library-load latency on the critical
   path of the first attention group.
 - one PSUM pool for the whole kernel (projections rotate through the
   same 8 (tag,slot) banks attention uses) so the phase transition has
   fine-grained deps instead of a pool-open full-PE-drain barrier; the
   small phase-2 SBUF pools are hoisted before phase 1 for the same
   reason.
 - prologue DMA triggers reordered: hst[0] first half + wv[0] first,
   weights next, cos/sin/masks last (the sync engine issues triggers
   serially at ~650ns each; the old order delayed the first matmul by
   ~5us).
 - V bias arrives pre-broadcast [128, 512] from the host (drops the
   startup ones-row matmul + psum pool + scalar copy).
"""

import os
import numpy as np
import ml_dtypes

B, S, H = 4, 2048, 2048
NH, NKV, HD = 8, 4, 256
WINDOW = 1024
ROPE_BASE = 10000.0
SCALE = 256.0 ** -0.5
LH, LKV = 4, 2          # local q-heads / kv-heads per core
NT = S // 128           # 16 seq tiles of 128
NG = S // 512           # 4 q-groups of 512
HALF = HD // 2          # 128

BF16 = ml_dtypes.bfloat16

LAST_EXEC_NS = None

_CACHE = {}


def _install_ntff_hook():
    import sys, types
    if "antenv.axon_hooks" in sys.modules:
        return
    try:
        import antenv
        from trn_agent_boot.trn_boot import _ntff_profile_via_ctypes
        hook = _ntff_profile_via_ctypes("/opt/axon/libaxon_pjrt.so")
        mod = types.ModuleType("antenv.axon_hooks")
        store = [hook]
        mod.set_axon_ntff_profile_hook = lambda h: store.__setitem__(0, h)
        mod.get_axon_ntff_profile_hook = lambda: store[0]
        sys.modules["antenv.axon_hooks"] = mod
        antenv.axon_hooks = mod
    except Exception:
        pass


def _build():
    if "nc" in _CACHE:
        return _CACHE["nc"]
    import concourse.bass as bass  # noqa: F401
    import concourse.mybir as mybir
    import concourse.tile as tile
    from concourse import bacc
    from collections import deque

    fp32 = mybir.dt.float32
    bf16 = mybir.dt.bfloat16
    Exp = mybir.ActivationFunctionType.Exp

    nc = bacc.Bacc("TRN2", target_bir_lowering=False, debug=False, num_devices=8)

    hsT_d = nc.dram_tensor("hsT", [H, S], bf16, kind="ExternalInput")
    wq_d = nc.dram_tensor("wq", [H, LH * HD], bf16, kind="ExternalInput")
    wk_d = nc.dram_tensor("wk", [H, LKV * HD], bf16, kind="ExternalInput")
    wv_d = nc.dram_tensor("wv", [H, LKV * HD], bf16, kind="ExternalInput")
    wo_d = nc.dram_tensor("wo", [LH * HD, H], bf16, kind="ExternalInput")
    bqT_d = nc.dram_tensor("bqT", [128, 2 * LH], fp32, kind="ExternalInput")
    bkT_d = nc.dram_tensor("bkT", [128, 2 * LKV], fp32, kind="ExternalInput")
    bvb_d = nc.dram_tensor("bvb", [128, LKV * HD], fp32, kind="ExternalInput")
    cosT_d = nc.dram_tensor("cosT", [HALF, S], bf16, kind="ExternalInput")
    sinT_d = nc.dram_tensor("sinT", [HALF, S], bf16, kind="ExternalInput")
    maskd_d = nc.dram_tensor("maskd", [128, 128], bf16, kind="ExternalInput")
    maskl_d = nc.dram_tensor("maskl", [128, 128], bf16, kind="ExternalInput")
    out_d = nc.dram_tensor("out", [S, H], fp32, kind="ExternalOutput")

    with tile.TileContext(nc) as tc:
        from contextlib import ExitStack
        with ExitStack() as ctx:
            persist = ctx.enter_context(tc.tile_pool(name="persist", bufs=1))

            # --- persistent tiles -------------------------------------------------
            qt = [persist.tile([128, S], bf16, tag=f"qt{i}", name=f"qt{i}") for i in range(2 * LH)]
            kt = [persist.tile([128, S], bf16, tag=f"kt{i}", name=f"kt{i}") for i in range(2 * LKV)]
            vt = [persist.tile([128, LKV * HD], bf16, tag=f"vt{i}", name=f"vt{i}") for i in range(NT)]

            cos_sb = persist.tile([HALF, S], bf16, tag="cos", name="cos")
            sin_sb = persist.tile([HALF, S], bf16, tag="sin", name="sin")
            maskd_sb = persist.tile([128, 128], bf16, tag="maskd", name="maskd")
            maskl_sb = persist.tile([128, 128], bf16, tag="maskl", name="maskl")
            bq_sb = persist.tile([128, 2 * LH], fp32, tag="bq", name="bq")
            bk_sb = persist.tile([128, 2 * LKV], fp32, tag="bk", name="bk")
            bvb_sb = persist.tile([128, LKV * HD], fp32, tag="bvb", name="bvb")
            ones_col = persist.tile([128, 1], bf16, tag="ones_col", name="ones_col")
            nc.vector.memset(ones_col[:], 1.0)

            # warm up the gpsimd partition_broadcast custom op during phase 1:
            # its first invocation pays a multi-us library-load latency that
            # would otherwise head-of-line block the softmax normalize chain
            # of the first attention group.
            warm_r = persist.tile([1, 512], fp32, tag="warm_r", name="warm_r")
            warm_b = persist.tile([128, 512], fp32, tag="warm_b", name="warm_b")
            nc.vector.memset(warm_r[:], 1.0)
            nc.gpsimd.partition_broadcast(warm_b[:], warm_r[:])

            # single PSUM pool for the whole kernel: projections rotate
            # through the same 8 (tag,slot) banks attention uses, so the
            # phase transition has fine-grained deps instead of a pool-open
            # barrier that waits for a full PE drain.
            ps2 = ctx.enter_context(tc.tile_pool(name="ps2", bufs=1, space="PSUM"))
            _ps_cnt = [0]

            def proj_ps():
                i = _ps_cnt[0] % 8
                _ps_cnt[0] += 1
                tag, bufs = ("big", 4) if i < 4 else (("a0", 2) if i < 6 else ("a1", 2))
                return ps2.tile([128, 512], fp32, tag=tag, name=tag, bufs=bufs)

            # --- phase 1: projections --------------------------------------------
            rpool = ctx.enter_context(tc.tile_pool(name="rpool", bufs=6))
            # phase-2 small pools hoisted here so their first use doesn't pay
            # an SBUF pool-open barrier at the phase transition
            epool = ctx.enter_context(tc.tile_pool(name="epool", bufs=7))
            espool = ctx.enter_context(tc.tile_pool(name="espool", bufs=2))
            rspool = ctx.enter_context(tc.tile_pool(name="rspool", bufs=2))
            with tc.tile_pool(name="wpool", bufs=1) as wpool, \
                 tc.tile_pool(name="hpool", bufs=18) as hpool:

                wq_t = [wpool.tile([128, LH * HD], bf16, tag=f"wq{t}", name=f"wq{t}") for t in range(NT)]
                wk_t = [wpool.tile([128, LKV * HD], bf16, tag=f"wk{t}", name=f"wk{t}") for t in range(NT)]
                wv_t = [wpool.tile([128, LKV * HD], bf16, tag=f"wv{t}", name=f"wv{t}") for t in range(NT)]

                # prologue DMA triggers: each engine issues one trigger per
                # ~650ns SERIALLY, so spread them across idle engine queues —
                # hst + wq on sync, wv (+bvb) on scalar, wk (+biases) on
                # gpsimd — so the V/K projections never outrun DMA supply.
                # Later-needed tensors (cos/sin/masks) trigger last.
                hst0 = []
                nc.scalar.dma_start(bvb_sb[:], bvb_d.ap())
                for t in range(NT):
                    nc.scalar.dma_start(wv_t[t][:], wv_d.ap()[t * 128:(t + 1) * 128, :])
                nc.gpsimd.dma_start(bq_sb[:], bqT_d.ap())
                nc.gpsimd.dma_start(bk_sb[:], bkT_d.ap())
                for t in range(NT):
                    nc.gpsimd.dma_start(wk_t[t][:], wk_d.ap()[t * 128:(t + 1) * 128, :])
                for t in range(NT):
                    htile = hpool.tile([128, 1024], bf16, tag="hs", name="hs")
                    if t == 0:
                        # split the first tile: V-proj's opening matmuls only
                        # read columns 0:512, so the PE can start sooner
                        nc.sync.dma_start(htile[:, 0:512], hsT_d.ap()[0:128, 0:512])
                        nc.sync.dma_start(htile[:, 512:1024], hsT_d.ap()[0:128, 512:1024])
                    else:
                        nc.sync.dma_start(htile[:], hsT_d.ap()[t * 128:(t + 1) * 128, 0:1024])
                    hst0.append(htile)
                for t in range(NT):
                    nc.sync.dma_start(wq_t[t][:], wq_d.ap()[t * 128:(t + 1) * 128, :])
                nc.sync.dma_start(cos_sb[:], cosT_d.ap())
                nc.sync.dma_start(sin_sb[:], sinT_d.ap())
                nc.sync.dma_start(maskd_sb[:], maskd_d.ap())
                nc.sync.dma_start(maskl_sb[:], maskl_d.ap())

                def rope_pair(x1, x2):
                    for sc in range(2):
                        sl = slice(sc * 1024, (sc + 1) * 1024)
                        c = cos_sb[:, sl]
                        s = sin_sb[:, sl]
                        t1 = rpool.tile([128, 1024], bf16, tag="rt", name="rt")
                        t2 = rpool.tile([128, 1024], bf16, tag="rt", name="rt")
                        t3 = rpool.tile([128, 1024], bf16, tag="rt", name="rt")
                        t4 = rpool.tile([128, 1024], bf16, tag="rt", name="rt")
                        nc.vector.tensor_mul(t1[:], x1[:, sl], c)
                        nc.vector.tensor_mul(t2[:], x2[:, sl], s)
                        nc.vector.tensor_mul(t3[:], x2[:, sl], c)
                        nc.vector.tensor_mul(t4[:], x1[:, sl], s)
                        nc.vector.tensor_sub(x1[:, sl], t1[:], t2[:])
                        nc.vector.tensor_add(x2[:, sl], t3[:], t4[:])

                for half in range(2):
                    s0 = half * 1024
                    if half == 0:
                        hst = hst0
                    else:
                        hst = []
                        for t in range(NT):
                            htile = hpool.tile([128, 1024], bf16, tag="hs", name="hs")
                            nc.sync.dma_start(htile[:], hsT_d.ap()[t * 128:(t + 1) * 128, s0:s0 + 1024])
                            hst.append(htile)

                    # t-major batches of 4 psum groups: the PE consumes weight/
                    # hidden tiles in DMA-arrival order instead of blocking on a
                    # full accumulation group.
                    order = ["v", "k", "q"]
                    for part in order:
                        if part == "v":
                            # V projection -> V natural [s, d]
                            for batch in range(2):
                                pss = [proj_ps() for _ in range(4)]
                                for t in range(NT):
                                    for gi in range(4):
                                        stl = batch * 4 + gi
                                        nc.tensor.matmul(
                                            pss[gi][:],
                                            hst[t][:, stl * 128:(stl + 1) * 128],
                                            wv_t[t][:],
                                            start=(t == 0), stop=(t == NT - 1))
                                for gi in range(4):
                                    st = half * 8 + batch * 4 + gi
                                    nc.vector.tensor_add(vt[st][:], pss[gi][:], bvb_sb[:])
                        elif part == "k":
                            # K projection -> kT, rope each kv pair as soon as complete
                            for kv in range(LKV):
                                pss = [proj_ps() for _ in range(4)]
                                grps = [(2 * kv + o, sc) for o in range(2) for sc in range(2)]
                                for t in range(NT):
                                    for gi, (ot, sc) in enumerate(grps):
                                        nc.tensor.matmul(
                                            pss[gi][:],
                                            wk_t[t][:, ot * 128:(ot + 1) * 128],
                                            hst[t][:, sc * 512:(sc + 1) * 512],
                                            start=(t == 0), stop=(t == NT - 1))
                                for gi, (ot, sc) in enumerate(grps):
                                    nc.vector.tensor_scalar_add(
                                        kt[ot][:, s0 + sc * 512: s0 + (sc + 1) * 512],
                                        pss[gi][:], bk_sb[:, ot:ot + 1])
                                if half == 1:
                                    rope_pair(kt[2 * kv], kt[2 * kv + 1])
                        else:
                            # Q projection -> qT, rope each head pair as soon as complete
                            for h in range(LH):
                                pss = [proj_ps() for _ in range(4)]
                                grps = [(2 * h + o, sc) for o in range(2) for sc in range(2)]
                                for t in range(NT):
                                    for gi, (ot, sc) in enumerate(grps):
                                        nc.tensor.matmul(
                                            pss[gi][:],
                                            wq_t[t][:, ot * 128:(ot + 1) * 128],
                                            hst[t][:, sc * 512:(sc + 1) * 512],
                                            start=(t == 0), stop=(t == NT - 1))
                                for gi, (ot, sc) in enumerate(grps):
                                    nc.vector.tensor_scalar_add(
                                        qt[ot][:, s0 + sc * 512: s0 + (sc + 1) * 512],
                                        pss[gi][:], bq_sb[:, ot:ot + 1])
                                if half == 1:
                                    rope_pair(qt[2 * h], qt[2 * h + 1])

            # --- phase 2: attention + O-projection -------------------------------
            with tc.tile_pool(name="wopool", bufs=1) as wopool, \
                 tc.tile_pool(name="anpool", bufs=2) as anpool, \
                 tc.tile_pool(name="obpool", bufs=4) as obpool:

                wo_t = [wopool.tile([128, H], bf16, tag=f"wo{t}", name=f"wo{t}") for t in range(2 * LH)]
                for t in range(2 * LH):
                    nc.sync.dma_start(wo_t[t][:], wo_d.ap()[t * 128:(t + 1) * 128, :])

                an = {}
                state = {}

                def block_range(g, t):
                    # valid q-subtiles for this k-tile: qj in [max(4g,t), min(4g+3,t+8)]
                    qjlo = max(4 * g, t)
                    qjhi = min(4 * g + 3, t + 8)
                    return (qjlo - 4 * g) * 128, (qjhi - qjlo + 1) * 128

                # flattened work stream: head-major over (h, g), k-tiles in
                # "full width first" border order within each group.
                items = []
                for h in range(LH):
                    for g in range(NG):
                        band = list(range(max(0, 4 * g - 8), 4 * g + 4))
                        border = [t for t in band if block_range(g, t)[1] == 512] + \
                                 [t for t in band if block_range(g, t)[1] < 512]
                        for ti, t in enumerate(border):
                            items.append((h, g, ti, t, len(border)))

                def emit_scores(h, g, ti, t):
                    kv = h // 2
                    q0 = g * 512
                    co, n = block_range(g, t)
                    if ti == 0:
                        state[(h, g)] = {
                            "a": [ps2.tile([128, 512], fp32, tag="a0", name="a0", bufs=2),
                                  ps2.tile([128, 512], fp32, tag="a1", name="a1", bufs=2)],
                            "esum": espool.tile([128, 512], bf16, tag="esum", name="esum"),
                        }
                    sc_ps = ps2.tile([128, 512], fp32, tag="big", name="big", bufs=4)
                    nc.tensor.matmul(
                        sc_ps[:, co:co + n],
                        kt[2 * kv][:, t * 128:(t + 1) * 128],
                        qt[2 * h][:, q0 + co:q0 + co + n],
                        start=True, stop=False)
                    nc.tensor.matmul(
                        sc_ps[:, co:co + n],
                        kt[2 * kv + 1][:, t * 128:(t + 1) * 128],
                        qt[2 * h + 1][:, q0 + co:q0 + co + n],
                        start=False, stop=True)
                    eT = epool.tile([128, 512], bf16, tag="e", name="e")
                    nc.scalar.activation(eT[:, co:co + n], sc_ps[:, co:co + n],
                                         Exp, scale=float(SCALE))
                    if 4 * g <= t:  # diagonal block is first in range
                        nc.vector.tensor_mul(eT[:, co:co + 128],
                                             eT[:, co:co + 128], maskd_sb[:])
                    if t + 8 <= 4 * g + 3:  # window-edge block is last in range
                        nc.vector.tensor_mul(eT[:, co + n - 128:co + n],
                                             eT[:, co + n - 128:co + n], maskl_sb[:])
                    # accumulate the softmax denominator on the DVE (same
                    # partial widths the old PE ones-matmuls used)
                    esum = state[(h, g)]["esum"]
                    if ti == 0:
                        nc.vector.tensor_scalar_add(esum[:], eT[:], 0.0)
                    else:
                        nc.vector.tensor_add(esum[:, co:co + n], esum[:, co:co + n],
                                             eT[:, co:co + n])
                    return eT

                def emit_pv(h, g, ti, t, nb, eT):
                    st = state[(h, g)]
                    first, last = ti == 0, ti == nb - 1
                    co, n = block_range(g, t)
                    kv = h // 2
                    for dh in range(2):
                        nc.tensor.matmul(
                            st["a"][dh][:, co:co + n],
                            vt[t][:, kv * 256 + dh * 128: kv * 256 + (dh + 1) * 128],
                            eT[:, co:co + n],
                            start=first, stop=last,
                            skip_group_check=True)

                def finish_a(h, g):
                    # cross-partition sum of esum via one N=512 ones-matmul
                    # (the only PE cost of the softmax denominator per group)
                    st = state[(h, g)]
                    s_ps = ps2.tile([128, 512], fp32, tag="big", name="big", bufs=4)
                    nc.tensor.matmul(s_ps[0:1, :], ones_col[:], st["esum"][:],
                                     start=True, stop=True)
                    st["s_ps"] = s_ps

                def finish_b(h, g):
                    # 1/sums, broadcast to all partitions on the idle gpsimd
                    st = state[(h, g)]
                    r_sb = rspool.tile([1, 512], fp32, tag="r", name="r")
                    nc.vector.reciprocal_approx_fast(r_sb[:], st["s_ps"][0:1, :])
                    b_sb = rspool.tile([128, 512], fp32, tag="b", name="b")
                    nc.gpsimd.partition_broadcast(b_sb[:], r_sb[:])
                    st["b_sb"] = b_sb

                def finish_c(h, g):
                    st = state.pop((h, g))
                    for dh in range(2):
                        a_n = anpool.tile([128, 512], bf16, tag=f"an{g}_{h}_{dh}",
                                          name=f"an{g}_{h}_{dh}", bufs=1)
                        nc.vector.tensor_mul(a_n[:], st["a"][dh][:], st["b_sb"][:])
                        an[(g, h, dh)] = a_n

                # scores run 2 k-tiles ahead of PV so the exp+mask chain never
                # stalls the PE; the group-finish DVE work is deferred a few
                # items so the in-order DVE queue doesn't block on gpsimd.
                pend = deque()
                fin = deque()
                def pump_fin():
                    while fin and fin[0][0] <= 0:
                        _, fn, hh, gg = fin.popleft()
                        fn(hh, gg)
                    for i in range(len(fin)):
                        e = fin[i]
                        fin[i] = (e[0] - 1, e[1], e[2], e[3])

                def after_pv(pi):
                    pump_fin()
                    if pi[2] == pi[4] - 1:  # last tile of its group
                        fin.append((1, finish_a, pi[0], pi[1]))
                        fin.append((2, finish_b, pi[0], pi[1]))
                        fin.append((4, finish_c, pi[0], pi[1]))

                for it in items:
                    h, g, ti, t, nb = it
                    eT = emit_scores(h, g, ti, t)
                    pend.append((h, g, ti, t, nb, eT))
                    if len(pend) > 5:
                        pi = pend.popleft()
                        emit_pv(*pi)
                        after_pv(pi)
                while pend:
                    pi = pend.popleft()
                    emit_pv(*pi)
                    after_pv(pi)
                while fin:
                    _, fn, hh, gg = fin.popleft()
                    fn(hh, gg)

                # O-projection, all groups at the end
                for g in range(NG):
                    for qjl in range(4):
                        for nch in range(4):
                            o_ps = ps2.tile([128, 512], fp32, tag="big", name="big", bufs=4)
                            idx = 0
                            for h in range(LH):
                                for dh in range(2):
                                    nc.tensor.matmul(
                                        o_ps[:],
                                        an[(g, h, dh)][:, qjl * 128:(qjl + 1) * 128],
                                        wo_t[2 * h + dh][:, nch * 512:(nch + 1) * 512],
                                        start=(idx == 0), stop=(idx == 2 * LH - 1))
                                    idx += 1
                            ob = obpool.tile([128, 512], fp32, tag="ob", name="ob")
                            nc.scalar.copy(ob[:], o_ps[:])
                            nc.sync.dma_start(
                                out_d.ap()[(4 * g + qjl) * 128:(4 * g + qjl + 1) * 128,
                                           nch * 512:(nch + 1) * 512],
                                ob[:])

    nc.compile()
    _CACHE["nc"] = nc
    return nc


def _prep_core_inputs(hidden_states, positions, Wq, bq, Wk, bk, Wv, bv, Wo):
    hs = np.asarray(hidden_states, dtype=np.float32)
    positions = np.asarray(positions)
    Wq = np.asarray(Wq, dtype=np.float32)
    Wk = np.asarray(Wk, dtype=np.float32)
    Wv = np.asarray(Wv, dtype=np.float32)
    Wo = np.asarray(Wo, dtype=np.float32)
    bq = np.asarray(bq, dtype=np.float32)
    bk = np.asarray(bk, dtype=np.float32)
    bv = np.asarray(bv, dtype=np.float32)

    p = np.arange(128, dtype=np.float32)
    inv_freq = (1.0 / (ROPE_BASE ** (p / 128.0)))[:, None]  # [128,1]

    maskd = np.zeros((128, 128), dtype=BF16)
    maskl = np.zeros((128, 128), dtype=BF16)
    ii = np.arange(128)[:, None]
    jj = np.arange(128)[None, :]
    maskd[ii <= jj] = 1.0
    maskl[ii > jj] = 1.0

    in_maps = []
    for c in range(8):
        b, hh = c // 2, c % 2
        freqs = positions[b][None, :].astype(np.float32) * inv_freq  # [128, S]
        bv_loc = bv[hh * 512:(hh + 1) * 512]
        in_maps.append({
            "hsT": np.ascontiguousarray(hs[b].T).astype(BF16),
            "wq": np.ascontiguousarray(Wq[:, hh * 1024:(hh + 1) * 1024]).astype(BF16),
            "wk": np.ascontiguousarray(Wk[:, hh * 512:(hh + 1) * 512]).astype(BF16),
            "wv": np.ascontiguousarray(Wv[:, hh * 512:(hh + 1) * 512]).astype(BF16),
            "wo": np.ascontiguousarray(Wo[hh * 1024:(hh + 1) * 1024, :]).astype(BF16),
            "bqT": np.ascontiguousarray(bq[hh * 1024:(hh + 1) * 1024].reshape(8, 128).T),
            "bkT": np.ascontiguousarray(bk[hh * 512:(hh + 1) * 512].reshape(4, 128).T),
            "bvb": np.ascontiguousarray(np.broadcast_to(bv_loc[None, :], (128, 512))).astype(np.float32),
            "cosT": np.cos(freqs).astype(BF16),
            "sinT": np.sin(freqs).astype(BF16),
            "maskd": maskd,
            "maskl": maskl,
        })
    return in_maps


def kernel(hidden_states, positions, Wq, bq, Wk, bk, Wv, bv, Wo):
    global LAST_EXEC_NS
    trace = bool(os.environ.get("PROBLEM_TRACE"))
    if trace:
        _install_ntff_hook()

    from concourse.bass_utils import run_bass_kernel_spmd

    nc = _build()
    in_maps = _prep_core_inputs(hidden_states, positions, Wq, bq, Wk, bk, Wv, bv, Wo)

    kwargs = {}
    if trace:
        kwargs["trace"] = True
        tdir = os.environ.get("PROBLEM_TRACE_DIR")
        if tdir:
            os.makedirs(tdir, exist_ok=True)
            kwargs["tmpdir"] = tdir
    res = run_bass_kernel_spmd(nc, in_maps, core_ids=list(range(8)), **kwargs)
    LAST_EXEC_NS = res.exec_time_ns

    out = np.empty((B, S, H), dtype=np.float32)
    for b in range(B):
        out[b] = res.results[2 * b]["out"] + res.results[2 * b + 1]["out"]
    return out


# revision 48
# speedup vs baseline: 1.0282x; 1.0282x over previous
"""Gemma2 sliding-window attention on 8 Trainium2 NeuronCores.

Sharding: data-parallel over batch (4) x tensor-parallel over heads (2).
Core c handles batch b = c//2 and head-half hh = c%2 (4 of 8 q-heads,
2 of 4 kv-heads). Each core computes a partial output [S, H] =
attn(local heads) @ Wo[local rows, :]; host sums the two partials per batch.

Device layout choice: everything is computed in "transposed" orientation.
hidden is fed as hsT [H, S]; projections produce qT/kT [d, s] and V [s, d];
scores are computed transposed sT [k, q] so the softmax denominator comes
from cross-partition reduction; PV produces attnT [d, q] which feeds the
O-projection as the stationary operand, yielding out [s, h] directly.

Optimizations vs the original baseline (519us -> ~456us):
 - softmax denominators mostly moved OFF the PE: eT tiles are accumulated
   into a per-group esum on the DVE (partial-width adds mirroring the old
   per-k-tile ones-matmul widths), leaving a single N=512 ones-matmul per
   group (~47k PE columns saved), then DVE reciprocal + gpsimd broadcast.
 - phase 2 is a single flattened (h,g,k-tile) work stream with a 5-deep
   scores-ahead-of-PV pipeline, so the scores->exp->mask chain latency is
   hidden and group boundaries don't bubble the PE. The sc_ps ring is 4
   banks (sums share the "big" ring, using partition row 0 only).
 - the group-finish chain (ones-matmul -> recip -> gpsimd broadcast ->
   normalize muls) is staged over deferred work items (delays 1/2/4 pops)
   so the in-order DVE queue never head-of-line blocks on gpsimd, while
   group G's normalize muls still land before group G+2 reuses its psum
   accumulator banks (the PV matmuls use skip_group_check, so only
   program order protects that WAR).
 - gpsimd partition_broadcast is warmed up during phase 1; its first
   invocation otherwise pays a ~6us library-load latency on the critical
   path of the first attention group.
 - one PSUM pool for the whole kernel (projections rotate through the
   same 8 (tag,slot) banks attention uses) so the phase transition has
   fine-grained deps instead of a pool-open full-PE-drain barrier; the
   small phase-2 SBUF pools are hoisted before phase 1 for the same
   reason.
 - prologue DMA triggers reordered: hst[0] first half + wv[0] first,
   weights next, cos/sin/masks last (the sync engine issues triggers
   serially at ~650ns each; the old order delayed the first matmul by
   ~5us).
 - V bias arrives pre-broadcast [128, 512] from the host (drops the
   startup ones-row matmul + psum pool + scalar copy).
"""

import os
import numpy as np
import ml_dtypes

B, S, H = 4, 2048, 2048
NH, NKV, HD = 8, 4, 256
WINDOW = 1024
ROPE_BASE = 10000.0
SCALE = 256.0 ** -0.5
LH, LKV = 4, 2          # local q-heads / kv-heads per core
NT = S // 128           # 16 seq tiles of 128
NG = S // 512           # 4 q-groups of 512
HALF = HD // 2          # 128

BF16 = ml_dtypes.bfloat16

LAST_EXEC_NS = None

_CACHE = {}


def _install_ntff_hook():
    import sys, types
    if "antenv.axon_hooks" in sys.modules:
        return
    try:
        import antenv
        from trn_agent_boot.trn_boot import _ntff_profile_via_ctypes
        hook = _ntff_profile_via_ctypes("/opt/axon/libaxon_pjrt.so")
        mod = types.ModuleType("antenv.axon_hooks")
        store = [hook]
        mod.set_axon_ntff_profile_hook = lambda h: store.__setitem__(0, h)
        mod.get_axon_ntff_profile_hook = lambda: store[0]
        sys.modules["antenv.axon_hooks"] = mod
        antenv.axon_hooks = mod
    except Exception:
        pass


def _build():
    if "nc" in _CACHE:
        return _CACHE["nc"]
    import concourse.bass as bass  # noqa: F401
    import concourse.mybir as mybir
    import concourse.tile as tile
    from concourse import bacc
    from collections import deque

    fp32 = mybir.dt.float32
    bf16 = mybir.dt.bfloat16
    Exp = mybir.ActivationFunctionType.Exp

    nc = bacc.Bacc("TRN2", target_bir_lowering=False, debug=False, num_devices=8)

    hsT_d = nc.dram_tensor("hsT", [H, S], bf16, kind="ExternalInput")
    wq_d = nc.dram_tensor("wq", [H, LH * HD], bf16, kind="ExternalInput")
    wk_d = nc.dram_tensor("wk", [H, LKV * HD], bf16, kind="ExternalInput")
    wv_d = nc.dram_tensor("wv", [H, LKV * HD], bf16, kind="ExternalInput")
    wo_d = nc.dram_tensor("wo", [LH * HD, H], bf16, kind="ExternalInput")
    bqT_d = nc.dram_tensor("bqT", [128, 2 * LH], fp32, kind="ExternalInput")
    bkT_d = nc.dram_tensor("bkT", [128, 2 * LKV], fp32, kind="ExternalInput")
    bvb_d = nc.dram_tensor("bvb", [128, LKV * HD], fp32, kind="ExternalInput")
    cosT_d = nc.dram_tensor("cosT", [HALF, S], bf16, kind="ExternalInput")
    sinT_d = nc.dram_tensor("sinT", [HALF, S], bf16, kind="ExternalInput")
    maskd_d = nc.dram_tensor("maskd", [128, 128], bf16, kind="ExternalInput")
    maskl_d = nc.dram_tensor("maskl", [128, 128], bf16, kind="ExternalInput")
    out_d = nc.dram_tensor("out", [S, H], fp32, kind="ExternalOutput")

    with tile.TileContext(nc) as tc:
        from contextlib import ExitStack
        with ExitStack() as ctx:
            persist = ctx.enter_context(tc.tile_pool(name="persist", bufs=1))

            # --- persistent tiles -------------------------------------------------
            qt = [persist.tile([128, S], bf16, tag=f"qt{i}", name=f"qt{i}") for i in range(2 * LH)]
            kt = [persist.tile([128, S], bf16, tag=f"kt{i}", name=f"kt{i}") for i in range(2 * LKV)]
            vt = [persist.tile([128, LKV * HD], bf16, tag=f"vt{i}", name=f"vt{i}") for i in range(NT)]

            cos_sb = persist.tile([HALF, S], bf16, tag="cos", name="cos")
            sin_sb = persist.tile([HALF, S], bf16, tag="sin", name="sin")
            maskd_sb = persist.tile([128, 128], bf16, tag="maskd", name="maskd")
            maskl_sb = persist.tile([128, 128], bf16, tag="maskl", name="maskl")
            bq_sb = persist.tile([128, 2 * LH], fp32, tag="bq", name="bq")
            bk_sb = persist.tile([128, 2 * LKV], fp32, tag="bk", name="bk")
            bvb_sb = persist.tile([128, LKV * HD], fp32, tag="bvb", name="bvb")
            ones_col = persist.tile([128, 1], bf16, tag="ones_col", name="ones_col")
            nc.vector.memset(ones_col[:], 1.0)

            # warm up the gpsimd partition_broadcast custom op during phase 1:
            # its first invocation pays a multi-us library-load latency that
            # would otherwise head-of-line block the softmax normalize chain
            # of the first attention group.
            warm_r = persist.tile([1, 512], fp32, tag="warm_r", name="warm_r")
            warm_b = persist.tile([128, 512], fp32, tag="warm_b", name="warm_b")
            nc.vector.memset(warm_r[:], 1.0)
            nc.gpsimd.partition_broadcast(warm_b[:], warm_r[:])

            # single PSUM pool for the whole kernel: projections rotate
            # through the same 8 (tag,slot) banks attention uses, so the
            # phase transition has fine-grained deps instead of a pool-open
            # barrier that waits for a full PE drain.
            ps2 = ctx.enter_context(tc.tile_pool(name="ps2", bufs=1, space="PSUM"))
            _ps_cnt = [0]

            def proj_ps():
                i = _ps_cnt[0] % 8
                _ps_cnt[0] += 1
                tag, bufs = ("big", 4) if i < 4 else (("a0", 2) if i < 6 else ("a1", 2))
                return ps2.tile([128, 512], fp32, tag=tag, name=tag, bufs=bufs)

            # --- phase 1: projections --------------------------------------------
            rpool = ctx.enter_context(tc.tile_pool(name="rpool", bufs=6))
            # phase-2 small pools hoisted here so their first use doesn't pay
            # an SBUF pool-open barrier at the phase transition
            epool = ctx.enter_context(tc.tile_pool(name="epool", bufs=7))
            espool = ctx.enter_context(tc.tile_pool(name="espool", bufs=2))
            rspool = ctx.enter_context(tc.tile_pool(name="rspool", bufs=2))
            with tc.tile_pool(name="wpool", bufs=1) as wpool, \
                 tc.tile_pool(name="hpool", bufs=18) as hpool:

                wq_t = [wpool.tile([128, LH * HD], bf16, tag=f"wq{t}", name=f"wq{t}") for t in range(NT)]
                wk_t = [wpool.tile([128, LKV * HD], bf16, tag=f"wk{t}", name=f"wk{t}") for t in range(NT)]
                wv_t = [wpool.tile([128, LKV * HD], bf16, tag=f"wv{t}", name=f"wv{t}") for t in range(NT)]

                # prologue DMA trigger order matters: the sync engine issues
                # one trigger per ~650ns, so the first V-proj inputs go first
                # and everything not needed until later (cos/sin/masks) last.
                hst0 = []
                for t in range(NT):
                    htile = hpool.tile([128, 1024], bf16, tag="hs", name="hs")
                    if t == 0:
                        # split the first tile: V-proj's opening matmuls only
                        # read columns 0:512, so the PE can start sooner
                        nc.sync.dma_start(htile[:, 0:512], hsT_d.ap()[0:128, 0:512])
                        nc.sync.dma_start(wv_t[0][:], wv_d.ap()[0:128, :])
                        nc.sync.dma_start(htile[:, 512:1024], hsT_d.ap()[0:128, 512:1024])
                    else:
                        nc.sync.dma_start(htile[:], hsT_d.ap()[t * 128:(t + 1) * 128, 0:1024])
                        nc.sync.dma_start(wv_t[t][:], wv_d.ap()[t * 128:(t + 1) * 128, :])
                    hst0.append(htile)
                    if t == 1:
                        nc.sync.dma_start(bvb_sb[:], bvb_d.ap())
                for t in range(NT):
                    nc.sync.dma_start(wk_t[t][:], wk_d.ap()[t * 128:(t + 1) * 128, :])
                nc.sync.dma_start(bq_sb[:], bqT_d.ap())
                nc.sync.dma_start(bk_sb[:], bkT_d.ap())
                for t in range(NT):
                    nc.sync.dma_start(wq_t[t][:], wq_d.ap()[t * 128:(t + 1) * 128, :])
                nc.sync.dma_start(cos_sb[:], cosT_d.ap())
                nc.sync.dma_start(sin_sb[:], sinT_d.ap())
                nc.sync.dma_start(maskd_sb[:], maskd_d.ap())
                nc.sync.dma_start(maskl_sb[:], maskl_d.ap())

                def rope_pair(x1, x2):
                    for sc in range(2):
                        sl = slice(sc * 1024, (sc + 1) * 1024)
                        c = cos_sb[:, sl]
                        s = sin_sb[:, sl]
                        t1 = rpool.tile([128, 1024], bf16, tag="rt", name="rt")
                        t2 = rpool.tile([128, 1024], bf16, tag="rt", name="rt")
                        t3 = rpool.tile([128, 1024], bf16, tag="rt", name="rt")
                        t4 = rpool.tile([128, 1024], bf16, tag="rt", name="rt")
                        nc.vector.tensor_mul(t1[:], x1[:, sl], c)
                        nc.vector.tensor_mul(t2[:], x2[:, sl], s)
                        nc.vector.tensor_mul(t3[:], x2[:, sl], c)
                        nc.vector.tensor_mul(t4[:], x1[:, sl], s)
                        nc.vector.tensor_sub(x1[:, sl], t1[:], t2[:])
                        nc.vector.tensor_add(x2[:, sl], t3[:], t4[:])

                for half in range(2):
                    s0 = half * 1024
                    if half == 0:
                        hst = hst0
                    else:
                        hst = []
                        for t in range(NT):
                            htile = hpool.tile([128, 1024], bf16, tag="hs", name="hs")
                            nc.sync.dma_start(htile[:], hsT_d.ap()[t * 128:(t + 1) * 128, s0:s0 + 1024])
                            hst.append(htile)

                    # t-major batches of 4 psum groups: the PE consumes weight/
                    # hidden tiles in DMA-arrival order instead of blocking on a
                    # full accumulation group.
                    order = ["v", "k", "q"]
                    for part in order:
                        if part == "v":
                            # V projection -> V natural [s, d]
                            for batch in range(2):
                                pss = [proj_ps() for _ in range(4)]
                                for t in range(NT):
                                    for gi in range(4):
                                        stl = batch * 4 + gi
                                        nc.tensor.matmul(
                                            pss[gi][:],
                                            hst[t][:, stl * 128:(stl + 1) * 128],
                                            wv_t[t][:],
                                            start=(t == 0), stop=(t == NT - 1))
                                for gi in range(4):
                                    st = half * 8 + batch * 4 + gi
                                    nc.vector.tensor_add(vt[st][:], pss[gi][:], bvb_sb[:])
                        elif part == "k":
                            # K projection -> kT, rope each kv pair as soon as complete
                            for kv in range(LKV):
                                pss = [proj_ps() for _ in range(4)]
                                grps = [(2 * kv + o, sc) for o in range(2) for sc in range(2)]
                                for t in range(NT):
                                    for gi, (ot, sc) in enumerate(grps):
                                        nc.tensor.matmul(
                                            pss[gi][:],
                                            wk_t[t][:, ot * 128:(ot + 1) * 128],
                                            hst[t][:, sc * 512:(sc + 1) * 512],
                                            start=(t == 0), stop=(t == NT - 1))
                                for gi, (ot, sc) in enumerate(grps):
                                    nc.vector.tensor_scalar_add(
                                        kt[ot][:, s0 + sc * 512: s0 + (sc + 1) * 512],
                                        pss[gi][:], bk_sb[:, ot:ot + 1])
                                if half == 1:
                                    rope_pair(kt[2 * kv], kt[2 * kv + 1])
                        else:
                            # Q projection -> qT, rope each head pair as soon as complete
                            for h in range(LH):
                                pss = [proj_ps() for _ in range(4)]
                                grps = [(2 * h + o, sc) for o in range(2) for sc in range(2)]
                                for t in range(NT):
                                    for gi, (ot, sc) in enumerate(grps):
                                        nc.tensor.matmul(
                                            pss[gi][:],
                                            wq_t[t][:, ot * 128:(ot + 1) * 128],
                                            hst[t][:, sc * 512:(sc + 1) * 512],
                                            start=(t == 0), stop=(t == NT - 1))
                                for gi, (ot, sc) in enumerate(grps):
                                    nc.vector.tensor_scalar_add(
                                        qt[ot][:, s0 + sc * 512: s0 + (sc + 1) * 512],
                                        pss[gi][:], bq_sb[:, ot:ot + 1])
                                if half == 1:
                                    rope_pair(qt[2 * h], qt[2 * h + 1])

            # --- phase 2: attention + O-projection -------------------------------
            with tc.tile_pool(name="wopool", bufs=1) as wopool, \
                 tc.tile_pool(name="anpool", bufs=2) as anpool, \
                 tc.tile_pool(name="obpool", bufs=4) as obpool:

                wo_t = [wopool.tile([128, H], bf16, tag=f"wo{t}", name=f"wo{t}") for t in range(2 * LH)]
                for t in range(2 * LH):
                    nc.sync.dma_start(wo_t[t][:], wo_d.ap()[t * 128:(t + 1) * 128, :])

                an = {}
                state = {}

                def block_range(g, t):
                    # valid q-subtiles for this k-tile: qj in [max(4g,t), min(4g+3,t+8)]
                    qjlo = max(4 * g, t)
                    qjhi = min(4 * g + 3, t + 8)
                    return (qjlo - 4 * g) * 128, (qjhi - qjlo + 1) * 128

                # flattened work stream: head-major over (h, g), k-tiles in
                # "full width first" border order within each group.
                items = []
                for h in range(LH):
                    for g in range(NG):
                        band = list(range(max(0, 4 * g - 8), 4 * g + 4))
                        border = [t for t in band if block_range(g, t)[1] == 512] + \
                                 [t for t in band if block_range(g, t)[1] < 512]
                        for ti, t in enumerate(border):
                            items.append((h, g, ti, t, len(border)))

                def emit_scores(h, g, ti, t):
                    kv = h // 2
                    q0 = g * 512
                    co, n = block_range(g, t)
                    if ti == 0:
                        state[(h, g)] = {
                            "a": [ps2.tile([128, 512], fp32, tag="a0", name="a0", bufs=2),
                                  ps2.tile([128, 512], fp32, tag="a1", name="a1", bufs=2)],
                            "esum": espool.tile([128, 512], bf16, tag="esum", name="esum"),
                        }
                    sc_ps = ps2.tile([128, 512], fp32, tag="big", name="big", bufs=4)
                    nc.tensor.matmul(
                        sc_ps[:, co:co + n],
                        kt[2 * kv][:, t * 128:(t + 1) * 128],
                        qt[2 * h][:, q0 + co:q0 + co + n],
                        start=True, stop=False)
                    nc.tensor.matmul(
                        sc_ps[:, co:co + n],
                        kt[2 * kv + 1][:, t * 128:(t + 1) * 128],
                        qt[2 * h + 1][:, q0 + co:q0 + co + n],
                        start=False, stop=True)
                    eT = epool.tile([128, 512], bf16, tag="e", name="e")
                    nc.scalar.activation(eT[:, co:co + n], sc_ps[:, co:co + n],
                                         Exp, scale=float(SCALE))
                    if 4 * g <= t:  # diagonal block is first in range
                        nc.vector.tensor_mul(eT[:, co:co + 128],
                                             eT[:, co:co + 128], maskd_sb[:])
                    if t + 8 <= 4 * g + 3:  # window-edge block is last in range
                        nc.vector.tensor_mul(eT[:, co + n - 128:co + n],
                                             eT[:, co + n - 128:co + n], maskl_sb[:])
                    # accumulate the softmax denominator on the DVE (same
                    # partial widths the old PE ones-matmuls used)
                    esum = state[(h, g)]["esum"]
                    if ti == 0:
                        nc.vector.tensor_scalar_add(esum[:], eT[:], 0.0)
                    else:
                        nc.vector.tensor_add(esum[:, co:co + n], esum[:, co:co + n],
                                             eT[:, co:co + n])
                    return eT

                def emit_pv(h, g, ti, t, nb, eT):
                    st = state[(h, g)]
                    first, last = ti == 0, ti == nb - 1
                    co, n = block_range(g, t)
                    kv = h // 2
                    for dh in range(2):
                        nc.tensor.matmul(
                            st["a"][dh][:, co:co + n],
                            vt[t][:, kv * 256 + dh * 128: kv * 256 + (dh + 1) * 128],
                            eT[:, co:co + n],
                            start=first, stop=last,
                            skip_group_check=True)

                def finish_a(h, g):
                    # cross-partition sum of esum via one N=512 ones-matmul
                    # (the only PE cost of the softmax denominator per group)
                    st = state[(h, g)]
                    s_ps = ps2.tile([128, 512], fp32, tag="big", name="big", bufs=4)
                    nc.tensor.matmul(s_ps[0:1, :], ones_col[:], st["esum"][:],
                                     start=True, stop=True)
                    st["s_ps"] = s_ps

                def finish_b(h, g):
                    # 1/sums, broadcast to all partitions on the idle gpsimd
                    st = state[(h, g)]
                    r_sb = rspool.tile([1, 512], fp32, tag="r", name="r")
                    nc.vector.reciprocal_approx_fast(r_sb[:], st["s_ps"][0:1, :])
                    b_sb = rspool.tile([128, 512], fp32, tag="b", name="b")
                    nc.gpsimd.partition_broadcast(b_sb[:], r_sb[:])
                    st["b_sb"] = b_sb

                def finish_c(h, g):
                    st = state.pop((h, g))
                    for dh in range(2):
                        a_n = anpool.tile([128, 512], bf16, tag=f"an{g}_{h}_{dh}",
                                          name=f"an{g}_{h}_{dh}", bufs=1)
                        nc.vector.tensor_mul(a_n[:], st["a"][dh][:], st["b_sb"][:])
                        an[(g, h, dh)] = a_n

                # scores run 2 k-tiles ahead of PV so the exp+mask chain never
                # stalls the PE; the group-finish DVE work is deferred a few
                # items so the in-order DVE queue doesn't block on gpsimd.
                pend = deque()
                fin = deque()
                def pump_fin():
                    while fin and fin[0][0] <= 0:
                        _, fn, hh, gg = fin.popleft()
                        fn(hh, gg)
                    for i in range(len(fin)):
                        e = fin[i]
                        fin[i] = (e[0] - 1, e[1], e[2], e[3])

                def after_pv(pi):
                    pump_fin()
                    if pi[2] == pi[4] - 1:  # last tile of its group
                        fin.append((1, finish_a, pi[0], pi[1]))
                        fin.append((2, finish_b, pi[0], pi[1]))
                        fin.append((4, finish_c, pi[0], pi[1]))

                for it in items:
                    h, g, ti, t, nb = it
                    eT = emit_scores(h, g, ti, t)
                    pend.append((h, g, ti, t, nb, eT))
                    if len(pend) > 5:
                        pi = pend.popleft()
                        emit_pv(*pi)
                        after_pv(pi)
                while pend:
                    pi = pend.popleft()
                    emit_pv(*pi)
                    after_pv(pi)
                while fin:
                    _, fn, hh, gg = fin.popleft()
                    fn(hh, gg)

                # O-projection, all groups at the end
                for g in range(NG):
                    for qjl in range(4):
                        for nch in range(4):
                            o_ps = ps2.tile([128, 512], fp32, tag="big", name="big", bufs=4)
                            idx = 0
                            for h in range(LH):
                                for dh in range(2):
                                    nc.tensor.matmul(
                                        o_ps[:],
                                        an[(g, h, dh)][:, qjl * 128:(qjl + 1) * 128],
                                        wo_t[2 * h + dh][:, nch * 512:(nch + 1) * 512],
                                        start=(idx == 0), stop=(idx == 2 * LH - 1))
                                    idx += 1
                            ob = obpool.tile([128, 512], fp32, tag="ob", name="ob")
                            nc.scalar.copy(ob[:], o_ps[:])
                            nc.sync.dma_start(
                                out_d.ap()[(4 * g + qjl) * 128:(4 * g + qjl + 1) * 128,
                                           nch * 512:(nch + 1) * 512],
                                ob[:])

    nc.compile()
    _CACHE["nc"] = nc
    return nc


def _prep_core_inputs(hidden_states, positions, Wq, bq, Wk, bk, Wv, bv, Wo):
    hs = np.asarray(hidden_states, dtype=np.float32)
    positions = np.asarray(positions)
    Wq = np.asarray(Wq, dtype=np.float32)
    Wk = np.asarray(Wk, dtype=np.float32)
    Wv = np.asarray(Wv, dtype=np.float32)
    Wo = np.asarray(Wo, dtype=np.float32)
    bq = np.asarray(bq, dtype=np.float32)
    bk = np.asarray(bk, dtype=np.float32)
    bv = np.asarray(bv, dtype=np.float32)

    p = np.arange(128, dtype=np.float32)
    inv_freq = (1.0 / (ROPE_BASE ** (p / 128.0)))[:, None]  # [128,1]

    maskd = np.zeros((128, 128), dtype=BF16)
    maskl = np.zeros((128, 128), dtype=BF16)
    ii = np.arange(128)[:, None]
    jj = np.arange(128)[None, :]
    maskd[ii <= jj] = 1.0
    maskl[ii > jj] = 1.0

    in_maps = []
    for c in range(8):
        b, hh = c // 2, c % 2
        freqs = positions[b][None, :].astype(np.float32) * inv_freq  # [128, S]
        bv_loc = bv[hh * 512:(hh + 1) * 512]
        in_maps.append({
            "hsT": np.ascontiguousarray(hs[b].T).astype(BF16),
            "wq": np.ascontiguousarray(Wq[:, hh * 1024:(hh + 1) * 1024]).astype(BF16),
            "wk": np.ascontiguousarray(Wk[:, hh * 512:(hh + 1) * 512]).astype(BF16),
            "wv": np.ascontiguousarray(Wv[:, hh * 512:(hh + 1) * 512]).astype(BF16),
            "wo": np.ascontiguousarray(Wo[hh * 1024:(hh + 1) * 1024, :]).astype(BF16),
            "bqT": np.ascontiguousarray(bq[hh * 1024:(hh + 1) * 1024].reshape(8, 128).T),
            "bkT": np.ascontiguousarray(bk[hh * 512:(hh + 1) * 512].reshape(4, 128).T),
            "bvb": np.ascontiguousarray(np.broadcast_to(bv_loc[None, :], (128, 512))).astype(np.float32),
            "cosT": np.cos(freqs).astype(BF16),
            "sinT": np.sin(freqs).astype(BF16),
            "maskd": maskd,
            "maskl": maskl,
        })
    return in_maps


def kernel(hidden_states, positions, Wq, bq, Wk, bk, Wv, bv, Wo):
    global LAST_EXEC_NS
    trace = bool(os.environ.get("PROBLEM_TRACE"))
    if trace:
        _install_ntff_hook()

    from concourse.bass_utils import run_bass_kernel_spmd

    nc = _build()
    in_maps = _prep_core_inputs(hidden_states, positions, Wq, bq, Wk, bk, Wv, bv, Wo)

    kwargs = {}
    if trace:
        kwargs["trace"] = True
        tdir = os.environ.get("PROBLEM_TRACE_DIR")
        if tdir:
            os.makedirs(tdir, exist_ok=True)
            kwargs["tmpdir"] = tdir
    res = run_bass_kernel_spmd(nc, in_maps, core_ids=list(range(8)), **kwargs)
    LAST_EXEC_NS = res.exec_time_ns

    out = np.empty((B, S, H), dtype=np.float32)
    for b in range(B):
        out[b] = res.results[2 * b]["out"] + res.results[2 * b + 1]["out"]
    return out
